# revision 8
# baseline (speedup 1.0000x reference)
"""Multi-head self-attention (B=4, T=2048, D=1024, H=16) on 8 TRN2 NeuronCores.

Sharding: batch x head-group. Core c owns batch b=c//2 and head group
g=c%2 (heads 8g..8g+7, i.e. model dims [512g, 512g+512)):
  - W_Q/W_K/W_V rows [512g, 512g+512) -> per-core q/k/v of shape [2048, 512]
  - causal attention for its 8 heads on its batch (block-skipped)
  - partial output projection through W_O columns [512g, 512g+512)
Host sums the 2 partial outputs per batch (row-parallel W_O reduction).

Layouts (on device, per core):
  x    [128, 8, 2048]   x^T for this batch, model dim on partitions (bf16)
  qT/kT [128, 4, 2048]  transposed q/k; head h lives at plane h//2,
                        rows 64*(h%2).. (bf16)
  vaug [128, 16, 520]   v token-major; per head [64 dims | ones col]
  scores^T tiles [128 k-tok, 2, 512 q] in PSUM; exp on ACT (bf16 out);
  causal masking is a post-exp 0/1 multiply on GpSimd (keeps DVE free);
  softmax denominator = ones-column row of the AV output; 1/denom via
  DVE reciprocal, broadcast across 64 partitions with a rank-1 matmul.

Emission interleaves projection chunk qc -> attention for q-chunk qc ->
output projection for those tokens, so ScalarE exp overlaps TensorE
projection matmuls and the PE stays HAM-warm.
"""

import os
import sys

import numpy as np

if "/opt/trn_rl_repo" not in sys.path:
    sys.path.insert(0, "/opt/trn_rl_repo")

import ml_dtypes

B, T, D, NH, DH = 4, 2048, 1024, 16, 64
MT = D // 128       # 8 model-dim tiles
N_CORES = 8
HPC = 8             # heads per core
DPC = 512           # model dims per core (head group)

_cache = {}


def _build_nc():
    from contextlib import ExitStack

    import concourse.mybir as mybir
    import concourse.tile as tile
    from concourse import bacc

    BF = mybir.dt.bfloat16
    F32 = mybir.dt.float32
    EXP = mybir.ActivationFunctionType.Exp
    LN = mybir.ActivationFunctionType.Ln

    nc = bacc.Bacc("TRN2", target_bir_lowering=False, debug=False)

    xT_d = nc.dram_tensor("xT", [MT, 128, T], BF, kind="ExternalInput")
    wq_d = nc.dram_tensor("wqT", [MT, 128, DPC], BF, kind="ExternalInput")
    wk_d = nc.dram_tensor("wkT", [MT, 128, DPC], BF, kind="ExternalInput")
    wv_d = nc.dram_tensor("wvT", [MT, 128, DPC], BF, kind="ExternalInput")
    wo_d = nc.dram_tensor("woT", [4, 128, D], BF, kind="ExternalInput")
    cm_d = nc.dram_tensor("cmask", [4, 128, 512], BF, kind="ExternalInput")
    out_d = nc.dram_tensor("out", [T, D], BF, kind="ExternalOutput")

    with tile.TileContext(nc) as tc, ExitStack() as ctx:
        pers = ctx.enter_context(tc.tile_pool(name="pers", bufs=1))
        xs = pers.tile([128, MT, T], BF)
        wq = pers.tile([128, MT, DPC], BF)
        wk = pers.tile([128, MT, DPC], BF)
        wv = pers.tile([128, MT, DPC], BF)
        wo = pers.tile([128, 4, D], BF)
        masks = pers.tile([128, 4, 512], BF)
        ones64 = pers.tile([1, 64], BF)
        qT = pers.tile([128, 4, T], BF)
        kT = pers.tile([128, 4, T], BF)
        vaug = pers.tile([128, 16, 520], BF)
        hoQ = [pers.tile([128, 4, 512], BF, tag=f"ho{qc}", name=f"ho{qc}")
               for qc in range(4)]

        nc.vector.memset(ones64, 1.0)
        nc.vector.memset(vaug, 1.0)
        for mt in range(MT):
            nc.sync.dma_start(out=wq[:, mt, :], in_=wq_d[mt])
            nc.sync.dma_start(out=wk[:, mt, :], in_=wk_d[mt])
            nc.sync.dma_start(out=wv[:, mt, :], in_=wv_d[mt])
            nc.sync.dma_start(out=xs[:, mt, :], in_=xT_d[mt])
        for i in range(4):
            nc.sync.dma_start(out=masks[:, i, :], in_=cm_d[i])
            nc.sync.dma_start(out=wo[:, i, :], in_=wo_d[i])

        pp = ctx.enter_context(tc.tile_pool(name="pp", bufs=2, space="PSUM"))
        sp = ctx.enter_context(tc.tile_pool(name="ps_s", bufs=2, space="PSUM"))
        avp = ctx.enter_context(tc.tile_pool(name="ps_av", bufs=2, space="PSUM"))
        ep = ctx.enter_context(tc.tile_pool(name="esb", bufs=4))
        nrm = ctx.enter_context(tc.tile_pool(name="nrm", bufs=4))
        osb = ctx.enter_context(tc.tile_pool(name="osb", bufs=3))

        for qc in range(4):
            qs = slice(qc * 512, (qc + 1) * 512)
            # ---- projection for token chunk qc ----
            for dt in range(4):
                ds_ = slice(dt * 128, (dt + 1) * 128)
                pq = pp.tile([128, 512], F32, tag="pp")
                for mt in range(MT):
                    nc.tensor.matmul(pq, wq[:, mt, ds_], xs[:, mt, qs],
                                     start=(mt == 0), stop=(mt == MT - 1))
                nc.vector.tensor_copy(out=qT[:, dt, qs], in_=pq)
                pk = pp.tile([128, 512], F32, tag="pp")
                for mt in range(MT):
                    nc.tensor.matmul(pk, wk[:, mt, ds_], xs[:, mt, qs],
                                     start=(mt == 0), stop=(mt == MT - 1))
                nc.vector.tensor_copy(out=kT[:, dt, qs], in_=pk)
            for tt in range(4):
                ts_ = slice((qc * 4 + tt) * 128, (qc * 4 + tt + 1) * 128)
                pv = pp.tile([128, 512], F32, tag="pp")
                for mt in range(MT):
                    nc.tensor.matmul(pv, xs[:, mt, ts_], wv[:, mt, :],
                                     start=(mt == 0), stop=(mt == MT - 1))
                # scatter 8 heads' 64-col blocks into the 65-wide slots
                nc.vector.tensor_copy(
                    out=vaug[:, qc * 4 + tt, :].rearrange(
                        "p (h e) -> p h e", h=HPC)[:, :, 0:64],
                    in_=pv[:].rearrange("p (h e) -> p h e", h=HPC))

            # ---- causal attention for q-chunk qc, all 8 heads ----
            ns = 2 * (qc + 1)          # k-supertiles of 256 tokens
            for h in range(HPC):
                hp = 64 * (h % 2)
                dt = h // 2
                pavh = avp.tile([65, 512], F32, tag="pav")
                for s in range(ns):
                    pss = sp.tile([128, 2, 512], F32)
                    for j in range(2):
                        kt = 2 * s + j
                        ko = kt * 128
                        nc.tensor.matmul(
                            pss[:, j, :],
                            kT[hp:hp + 64, dt, ko:ko + 128],
                            qT[hp:hp + 64, dt, qs],
                            start=True, stop=True)
                    ex = ep.tile([128, 2, 512], BF)
                    nc.scalar.activation(out=ex[:], in_=pss[:], func=EXP,
                                         scale=0.125)
                    if s >= 2 * qc:    # diagonal band: zero future positions
                        j0 = 2 * (s - 2 * qc)
                        exm = ep.tile([128, 2, 512], BF, tag="exm")
                        nc.vector.tensor_mul(exm[:], ex[:],
                                             masks[:, j0:j0 + 2, :])
                        ex = exm
                    for j in range(2):
                        kt = 2 * s + j
                        nc.tensor.matmul(
                            pavh,
                            vaug[:, kt, 65 * h:65 * h + 65],
                            ex[:, j, :],
                            start=(s == 0 and j == 0),
                            stop=(s == ns - 1 and j == 1))
                lnd = nrm.tile([1, 512], F32)
                nc.scalar.activation(out=lnd, in_=pavh[64:65, :], func=LN)
                inv = nrm.tile([1, 512], BF)
                with nc.allow_low_precision(
                        reason="softmax 1/denom via exp(-ln d)"):
                    nc.scalar.activation(out=inv, in_=lnd, func=EXP,
                                         scale=-1.0)
                pbc = pp.tile([64, 512], F32, tag="pp")
                nc.tensor.matmul(pbc, ones64[:], inv[:], start=True, stop=True)
                invb = nrm.tile([64, 512], F32)
                nc.vector.tensor_copy(out=invb, in_=pbc)
                nc.vector.tensor_mul(hoQ[qc][hp:hp + 64, dt, :],
                                     pavh[0:64, :], invb)

            # ---- partial output projection for tokens of chunk qc ----
            for tt in range(4):
                to = qc * 512 + tt * 128
                for oc in range(2):
                    po = pp.tile([128, 512], F32, tag="pp")
                    for dt in range(4):
                        nc.tensor.matmul(
                            po,
                            hoQ[qc][:, dt, tt * 128:(tt + 1) * 128],
                            wo[:, dt, oc * 512:(oc + 1) * 512],
                            start=(dt == 0), stop=(dt == 3))
                    ost = osb.tile([128, 512], BF)
                    nc.vector.tensor_copy(out=ost, in_=po)
                    nc.sync.dma_start(
                        out=out_d[to:to + 128, oc * 512:(oc + 1) * 512],
                        in_=ost)
    nc.compile()
    return nc


def _get_nc():
    if "nc" not in _cache:
        _cache["nc"] = _build_nc()
    return _cache["nc"]


def _bf(a):
    return np.ascontiguousarray(a, dtype=np.float32).astype(ml_dtypes.bfloat16)


def make_in_maps(x, W_Q, W_K, W_V, W_O):
    x = np.asarray(x, np.float32)
    cmask = np.zeros((4, 128, 512), dtype=np.float32)
    for t in range(4):
        for kp in range(128):
            cmask[t, kp, t * 128 + kp:] = 1.0
    cmask = cmask.astype(ml_dtypes.bfloat16)
    in_maps = []
    for c in range(N_CORES):
        b, g = c // 2, c % 2
        rs = slice(g * DPC, (g + 1) * DPC)
        in_maps.append({
            "xT": _bf(x[b].T).reshape(MT, 128, T),
            "wqT": _bf(W_Q[rs, :].T).reshape(MT, 128, DPC),
            "wkT": _bf(W_K[rs, :].T).reshape(MT, 128, DPC),
            "wvT": _bf(W_V[rs, :].T).reshape(MT, 128, DPC),
            "woT": _bf(W_O[:, rs].T).reshape(4, 128, D),
            "cmask": cmask,
        })
    return in_maps


def _ensure_ntff_hook():
    """Install antenv.axon_hooks shim (missing in this image) so
    run_bass_kernel_spmd(trace=True) can capture NTFF profiles."""
    try:
        from antenv import axon_hooks  # noqa: F401
        return True
    except ImportError:
        pass
    try:
        import contextlib
        import ctypes
        import types

        import antenv

        so_path = "/opt/axon/libaxon_pjrt.so"
        lib = ctypes.CDLL(so_path)
        if not hasattr(lib, "axon_start_nrt_profile"):
            return False
        lib.axon_start_nrt_profile.argtypes = [
            ctypes.POINTER(ctypes.c_int64), ctypes.c_size_t]
        lib.axon_start_nrt_profile.restype = ctypes.c_int64
        lib.axon_stop_nrt_profile.argtypes = [ctypes.c_char_p]
        lib.axon_stop_nrt_profile.restype = ctypes.c_int64

        @contextlib.contextmanager
        def _hook(output_dir, device_ids):
            import jax
            jax.devices()
            if device_ids:
                ids = (ctypes.c_int64 * len(device_ids))(*device_ids)
                rc = lib.axon_start_nrt_profile(ids, len(device_ids))
            else:
                rc = lib.axon_start_nrt_profile(None, 0)
            if rc != 0:
                raise RuntimeError(f"axon_start_nrt_profile rc={rc}")
            try:
                yield
            finally:
                n = lib.axon_stop_nrt_profile(str(output_dir).encode())
                print(f"ntff profile: {n} file(s) -> {output_dir}",
                      file=sys.stderr)

        mod = types.ModuleType("antenv.axon_hooks")
        mod.get_axon_ntff_profile_hook = lambda: _hook
        mod.set_axon_ntff_profile_hook = lambda h: None
        sys.modules["antenv.axon_hooks"] = mod
        antenv.axon_hooks = mod
        return True
    except Exception as e:  # pragma: no cover
        print(f"ntff hook install failed: {e}", file=sys.stderr)
        return False


def bench_pjrt(in_maps, n_iters=8):
    """Run the SPMD program with device-resident inputs; return (results,
    per-iter wall times)."""
    import time

    import jax
    import concourse.mybir as mybir
    from jax.sharding import Mesh, NamedSharding, PartitionSpec
    from jax.experimental.shard_map import shard_map
    from concourse import bass2jax

    nc = _get_nc()
    bass2jax.install_neuronx_cc_hook()

    part_name = nc.partition_id_tensor.name if nc.partition_id_tensor else None
    in_names, out_names, out_avals, zero_outs = [], [], [], []
    for alloc in nc.m.functions[0].allocations:
        if not isinstance(alloc, mybir.MemoryLocationSet):
            continue
        name = alloc.memorylocations[0].name
        if alloc.kind == "ExternalInput":
            if name != part_name:
                in_names.append(name)
        elif alloc.kind == "ExternalOutput":
            shape = tuple(alloc.tensor_shape)
            dtype = mybir.dt.np(alloc.dtype)
            out_names.append(name)
            out_avals.append(jax.core.ShapedArray(shape, dtype))
            zero_outs.append(np.zeros(shape, dtype))
    n_params = len(in_names)
    all_names = in_names + out_names
    if part_name is not None:
        all_names = all_names + [part_name]

    def _body(*args):
        operands = list(args)
        if part_name is not None:
            operands.append(bass2jax.partition_id_tensor())
        outs = bass2jax._bass_exec_p.bind(
            *operands,
            out_avals=tuple(out_avals),
            in_names=tuple(all_names),
            out_names=tuple(out_names),
            lowering_input_output_aliases=(),
            sim_require_finite=True,
            sim_require_nnan=True,
            nc=nc,
        )
        return tuple(outs)

    n_cores = len(in_maps)
    devices = jax.devices()[:n_cores]
    mesh = Mesh(np.asarray(devices), ("core",))
    donate = tuple(range(n_params, n_params + len(out_names)))
    sharded = jax.jit(
        shard_map(_body, mesh=mesh,
                  in_specs=(PartitionSpec("core"),) * (n_params + len(out_names)),
                  out_specs=(PartitionSpec("core"),) * len(out_names),
                  check_rep=False),
        donate_argnums=donate, keep_unused=True)

    concat_in = [
        np.concatenate([np.asarray(in_maps[c][k]) for c in range(n_cores)],
                       axis=0) for k in in_names]
    concat_zeros = [np.zeros((n_cores * z.shape[0], *z.shape[1:]), z.dtype)
                    for z in zero_outs]
    sh = NamedSharding(mesh, PartitionSpec("core"))
    dev_in = [jax.device_put(a, sh) for a in concat_in]
    outs = sharded(*dev_in, *[jax.device_put(z, sh) for z in concat_zeros])
    jax.block_until_ready(outs)
    first = [np.asarray(o) for o in outs]

    times = []
    for _ in range(n_iters):
        t0 = time.perf_counter()
        outs = sharded(*dev_in, *outs)
        jax.block_until_ready(outs)
        times.append(time.perf_counter() - t0)

    results = [
        {name: first[i].reshape(n_cores, *out_avals[i].shape)[c]
         for i, name in enumerate(out_names)}
        for c in range(n_cores)
    ]
    return results, times


def _gather(results):
    out = np.zeros((B, T, D), dtype=np.float32)
    for c in range(N_CORES):
        out[c // 2] += np.asarray(results[c]["out"], dtype=np.float32)
    return out


def kernel(x, W_Q, W_K, W_V, W_O):
    import concourse.bass_utils as bass_utils

    x = np.asarray(x, dtype=np.float32)
    in_maps = make_in_maps(x, np.asarray(W_Q, np.float32),
                           np.asarray(W_K, np.float32),
                           np.asarray(W_V, np.float32),
                           np.asarray(W_O, np.float32))
    nc = _get_nc()
    trace = bool(int(os.environ.get("MHSA_TRACE", "0")))
    tmpdir = None
    if trace:
        trace = _ensure_ntff_hook()
    if trace:
        import tempfile
        tmpdir = tempfile.mkdtemp(prefix="mhsa_ntff_")
        _cache["trace_dir"] = tmpdir
        # no cloud creds in this container; keep artifacts local
        bass_utils.upload_artifacts = lambda d: f"local://{d}"
    res = bass_utils.run_bass_kernel_spmd(
        nc, in_maps, list(range(N_CORES)), trace=trace, tmpdir=tmpdir)
    _cache["last_results"] = res
    return _gather(res.results)


# revision 20
# speedup vs baseline: 1.3546x; 1.3546x over previous
"""Multi-head self-attention (B=4, T=2048, D=1024, H=16) on 8 TRN2 NeuronCores.

Sharding: batch x head-group. Core c owns batch b=c//2 and head group
g=c%2 (heads 8g..8g+7, i.e. model dims [512g, 512g+512)):
  - W_Q/W_K/W_V rows [512g, 512g+512) -> per-core q/k/v of shape [2048, 512]
  - causal attention for its 8 heads on its batch (block-skipped)
  - partial output projection through W_O columns [512g, 512g+512)
Host sums the 2 partial outputs per batch (row-parallel W_O reduction).

Layouts (on device, per core):
  x    [128, 8, 2048]   x^T for this batch, model dim on partitions (bf16)
  qT/kT [128, 4, 2048]  transposed q/k; head h lives at plane h//2,
                        rows 64*(h%2).. (bf16)
  vaug [128, 16, 520]   v token-major; per head [64 dims | ones col]
  scores^T tiles [128 k-tok, 2, 512 q] in PSUM; exp on ACT (bf16 out);
  causal masking is a post-exp 0/1 multiply on GpSimd (keeps DVE free);
  softmax denominator = ones-column row of the AV output; 1/denom via
  DVE reciprocal, broadcast across 64 partitions with a rank-1 matmul.

Emission interleaves projection chunk qc -> attention for q-chunk qc ->
output projection for those tokens, so ScalarE exp overlaps TensorE
projection matmuls and the PE stays HAM-warm.
"""

import os
import sys

import numpy as np

if "/opt/trn_rl_repo" not in sys.path:
    sys.path.insert(0, "/opt/trn_rl_repo")

import ml_dtypes

B, T, D, NH, DH = 4, 2048, 1024, 16, 64
MT = D // 128       # 8 model-dim tiles
N_CORES = 8
HPC = 8             # heads per core
DPC = 512           # model dims per core (head group)

_cache = {}


def _build_nc():
    from contextlib import ExitStack

    import concourse.mybir as mybir
    import concourse.tile as tile
    from concourse import bacc

    BF = mybir.dt.bfloat16
    F32 = mybir.dt.float32
    EXP = mybir.ActivationFunctionType.Exp
    LN = mybir.ActivationFunctionType.Ln

    nc = bacc.Bacc("TRN2", target_bir_lowering=False, debug=False)

    xT_d = nc.dram_tensor("xT", [MT, 128, T], BF, kind="ExternalInput")
    wq_d = nc.dram_tensor("wqT", [MT, 128, DPC], BF, kind="ExternalInput")
    wk_d = nc.dram_tensor("wkT", [MT, 128, DPC], BF, kind="ExternalInput")
    wv_d = nc.dram_tensor("wvT", [MT, 128, DPC], BF, kind="ExternalInput")
    wo_d = nc.dram_tensor("woT", [4, 128, D], BF, kind="ExternalInput")
    cm_d = nc.dram_tensor("cmask", [4, 128, 512], BF, kind="ExternalInput")
    out_d = nc.dram_tensor("out", [T, D], BF, kind="ExternalOutput")

    with tile.TileContext(nc) as tc, ExitStack() as ctx:
        pers = ctx.enter_context(tc.tile_pool(name="pers", bufs=1))
        xs = pers.tile([128, MT, T], BF)
        wq = pers.tile([128, MT, DPC], BF)
        wk = pers.tile([128, MT, DPC], BF)
        wv = pers.tile([128, MT, DPC], BF)
        wo = pers.tile([128, 4, D], BF)
        masks = pers.tile([128, 4, 512], BF)
        ones64 = pers.tile([1, 64], F32)
        qT = pers.tile([128, 4, T], BF)
        kT = pers.tile([128, 4, T], BF)
        vaug = pers.tile([128, 16, 520], BF)
        hoQ = [pers.tile([128, 4, 512], BF, tag=f"ho{qc}", name=f"ho{qc}")
               for qc in range(4)]

        nc.vector.memset(ones64, 1.0)
        nc.vector.memset(vaug, 1.0)
        for mt in range(MT):
            nc.sync.dma_start(out=wq[:, mt, :], in_=wq_d[mt])
            nc.sync.dma_start(out=wk[:, mt, :], in_=wk_d[mt])
            nc.sync.dma_start(out=wv[:, mt, :], in_=wv_d[mt])
            nc.sync.dma_start(out=xs[:, mt, :], in_=xT_d[mt])
        for i in range(4):
            nc.sync.dma_start(out=masks[:, i, :], in_=cm_d[i])
            nc.sync.dma_start(out=wo[:, i, :], in_=wo_d[i])

        pp = ctx.enter_context(tc.tile_pool(name="pp", bufs=2, space="PSUM"))
        sp = ctx.enter_context(tc.tile_pool(name="ps_s", bufs=2, space="PSUM"))
        avp = ctx.enter_context(tc.tile_pool(name="ps_av", bufs=2, space="PSUM"))
        ep = ctx.enter_context(tc.tile_pool(name="esb", bufs=4))
        nrm = ctx.enter_context(tc.tile_pool(name="nrm", bufs=4))
        osb = ctx.enter_context(tc.tile_pool(name="osb", bufs=3))

        for qc in range(4):
            qs = slice(qc * 512, (qc + 1) * 512)
            # ---- projection for token chunk qc ----
            for dt in range(4):
                ds_ = slice(dt * 128, (dt + 1) * 128)
                pq = pp.tile([128, 512], F32, tag="pp")
                for mt in range(MT):
                    nc.tensor.matmul(pq, wq[:, mt, ds_], xs[:, mt, qs],
                                     start=(mt == 0), stop=(mt == MT - 1))
                nc.vector.tensor_copy(out=qT[:, dt, qs], in_=pq)
                pk = pp.tile([128, 512], F32, tag="pp")
                for mt in range(MT):
                    nc.tensor.matmul(pk, wk[:, mt, ds_], xs[:, mt, qs],
                                     start=(mt == 0), stop=(mt == MT - 1))
                nc.vector.tensor_copy(out=kT[:, dt, qs], in_=pk)
            for tt in range(4):
                ts_ = slice((qc * 4 + tt) * 128, (qc * 4 + tt + 1) * 128)
                pv = pp.tile([128, 512], F32, tag="pp")
                for mt in range(MT):
                    nc.tensor.matmul(pv, xs[:, mt, ts_], wv[:, mt, :],
                                     start=(mt == 0), stop=(mt == MT - 1))
                # scatter 8 heads' 64-col blocks into the 65-wide slots
                nc.vector.tensor_copy(
                    out=vaug[:, qc * 4 + tt, :].rearrange(
                        "p (h e) -> p h e", h=HPC)[:, :, 0:64],
                    in_=pv[:].rearrange("p (h e) -> p h e", h=HPC))

            # ---- causal attention for q-chunk qc, all 8 heads ----
            ns = 2 * (qc + 1)          # k-supertiles of 256 tokens
            for h in range(HPC):
                hp = 64 * (h % 2)
                dt = h // 2
                pavh = avp.tile([65, 512], F32, tag="pav")
                for s in range(ns):
                    pss = sp.tile([128, 2, 512], F32)
                    for j in range(2):
                        kt = 2 * s + j
                        ko = kt * 128
                        nc.tensor.matmul(
                            pss[:, j, :],
                            kT[hp:hp + 64, dt, ko:ko + 128],
                            qT[hp:hp + 64, dt, qs],
                            start=True, stop=True)
                    ex = ep.tile([128, 2, 512], BF)
                    nc.scalar.activation(out=ex[:], in_=pss[:], func=EXP,
                                         scale=0.125)
                    if s >= 2 * qc:    # diagonal band: zero future positions
                        j0 = 2 * (s - 2 * qc)
                        exm = ep.tile([128, 2, 512], BF, tag="exm")
                        nc.vector.tensor_mul(exm[:], ex[:],
                                             masks[:, j0:j0 + 2, :])
                        ex = exm
                    for j in range(2):
                        kt = 2 * s + j
                        nc.tensor.matmul(
                            pavh,
                            vaug[:, kt, 65 * h:65 * h + 65],
                            ex[:, j, :],
                            start=(s == 0 and j == 0),
                            stop=(s == ns - 1 and j == 1))
                # 1/denominator: single-pass DVE approx (18 bits, plenty
                # for the 2e-2 gate), broadcast across 64 partitions via a
                # rank-1 matmul, then scale the head output.
                dsb = nrm.tile([1, 512], F32)
                nc.vector.tensor_copy(out=dsb, in_=pavh[64:65, :])
                inv = nrm.tile([1, 512], F32)
                nc.vector.reciprocal_approx_fast(out=inv, in_=dsb)
                pbc = pp.tile([64, 512], F32, tag="pp")
                nc.tensor.matmul(pbc, ones64[:], inv[:], start=True,
                                 stop=True)
                invb = nrm.tile([64, 512], F32)
                nc.vector.tensor_copy(out=invb, in_=pbc)
                nc.vector.tensor_mul(hoQ[qc][hp:hp + 64, dt, :],
                                     pavh[0:64, :], invb)

            # ---- partial output projection for tokens of chunk qc ----
            for tt in range(4):
                to = qc * 512 + tt * 128
                for oc in range(2):
                    po = pp.tile([128, 512], F32, tag="pp")
                    for dt in range(4):
                        nc.tensor.matmul(
                            po,
                            hoQ[qc][:, dt, tt * 128:(tt + 1) * 128],
                            wo[:, dt, oc * 512:(oc + 1) * 512],
                            start=(dt == 0), stop=(dt == 3))
                    ost = osb.tile([128, 512], BF)
                    nc.vector.tensor_copy(out=ost, in_=po)
                    nc.sync.dma_start(
                        out=out_d[to:to + 128, oc * 512:(oc + 1) * 512],
                        in_=ost)
    nc.compile()
    return nc


def _get_nc():
    if "nc" not in _cache:
        _cache["nc"] = _build_nc()
    return _cache["nc"]


def _bf(a):
    return np.ascontiguousarray(a, dtype=np.float32).astype(ml_dtypes.bfloat16)


def make_in_maps(x, W_Q, W_K, W_V, W_O):
    x = np.asarray(x, np.float32)
    cmask = np.zeros((4, 128, 512), dtype=np.float32)
    for t in range(4):
        for kp in range(128):
            cmask[t, kp, t * 128 + kp:] = 1.0
    cmask = cmask.astype(ml_dtypes.bfloat16)
    in_maps = []
    for c in range(N_CORES):
        b, g = c // 2, c % 2
        rs = slice(g * DPC, (g + 1) * DPC)
        in_maps.append({
            "xT": _bf(x[b].T).reshape(MT, 128, T),
            "wqT": _bf(W_Q[rs, :].T).reshape(MT, 128, DPC),
            "wkT": _bf(W_K[rs, :].T).reshape(MT, 128, DPC),
            "wvT": _bf(W_V[rs, :].T).reshape(MT, 128, DPC),
            "woT": _bf(W_O[:, rs].T).reshape(4, 128, D),
            "cmask": cmask,
        })
    return in_maps


def _ensure_ntff_hook():
    """Install antenv.axon_hooks shim (missing in this image) so
    run_bass_kernel_spmd(trace=True) can capture NTFF profiles."""
    try:
        from antenv import axon_hooks  # noqa: F401
        return True
    except ImportError:
        pass
    try:
        import contextlib
        import ctypes
        import types

        import antenv

        so_path = "/opt/axon/libaxon_pjrt.so"
        lib = ctypes.CDLL(so_path)
        if not hasattr(lib, "axon_start_nrt_profile"):
            return False
        lib.axon_start_nrt_profile.argtypes = [
            ctypes.POINTER(ctypes.c_int64), ctypes.c_size_t]
        lib.axon_start_nrt_profile.restype = ctypes.c_int64
        lib.axon_stop_nrt_profile.argtypes = [ctypes.c_char_p]
        lib.axon_stop_nrt_profile.restype = ctypes.c_int64

        @contextlib.contextmanager
        def _hook(output_dir, device_ids):
            import jax
            jax.devices()
            if device_ids:
                ids = (ctypes.c_int64 * len(device_ids))(*device_ids)
                rc = lib.axon_start_nrt_profile(ids, len(device_ids))
            else:
                rc = lib.axon_start_nrt_profile(None, 0)
            if rc != 0:
                raise RuntimeError(f"axon_start_nrt_profile rc={rc}")
            try:
                yield
            finally:
                n = lib.axon_stop_nrt_profile(str(output_dir).encode())
                print(f"ntff profile: {n} file(s) -> {output_dir}",
                      file=sys.stderr)

        mod = types.ModuleType("antenv.axon_hooks")
        mod.get_axon_ntff_profile_hook = lambda: _hook
        mod.set_axon_ntff_profile_hook = lambda h: None
        sys.modules["antenv.axon_hooks"] = mod
        antenv.axon_hooks = mod
        return True
    except Exception as e:  # pragma: no cover
        print(f"ntff hook install failed: {e}", file=sys.stderr)
        return False


def bench_pjrt(in_maps, n_iters=8):
    """Run the SPMD program with device-resident inputs; return (results,
    per-iter wall times)."""
    import time

    import jax
    import concourse.mybir as mybir
    from jax.sharding import Mesh, NamedSharding, PartitionSpec
    from jax.experimental.shard_map import shard_map
    from concourse import bass2jax

    nc = _get_nc()
    bass2jax.install_neuronx_cc_hook()

    part_name = nc.partition_id_tensor.name if nc.partition_id_tensor else None
    in_names, out_names, out_avals, zero_outs = [], [], [], []
    for alloc in nc.m.functions[0].allocations:
        if not isinstance(alloc, mybir.MemoryLocationSet):
            continue
        name = alloc.memorylocations[0].name
        if alloc.kind == "ExternalInput":
            if name != part_name:
                in_names.append(name)
        elif alloc.kind == "ExternalOutput":
            shape = tuple(alloc.tensor_shape)
            dtype = mybir.dt.np(alloc.dtype)
            out_names.append(name)
            out_avals.append(jax.core.ShapedArray(shape, dtype))
            zero_outs.append(np.zeros(shape, dtype))
    n_params = len(in_names)
    all_names = in_names + out_names
    if part_name is not None:
        all_names = all_names + [part_name]

    def _body(*args):
        operands = list(args)
        if part_name is not None:
            operands.append(bass2jax.partition_id_tensor())
        outs = bass2jax._bass_exec_p.bind(
            *operands,
            out_avals=tuple(out_avals),
            in_names=tuple(all_names),
            out_names=tuple(out_names),
            lowering_input_output_aliases=(),
            sim_require_finite=True,
            sim_require_nnan=True,
            nc=nc,
        )
        return tuple(outs)

    n_cores = len(in_maps)
    devices = jax.devices()[:n_cores]
    mesh = Mesh(np.asarray(devices), ("core",))
    donate = tuple(range(n_params, n_params + len(out_names)))
    sharded = jax.jit(
        shard_map(_body, mesh=mesh,
                  in_specs=(PartitionSpec("core"),) * (n_params + len(out_names)),
                  out_specs=(PartitionSpec("core"),) * len(out_names),
                  check_rep=False),
        donate_argnums=donate, keep_unused=True)

    concat_in = [
        np.concatenate([np.asarray(in_maps[c][k]) for c in range(n_cores)],
                       axis=0) for k in in_names]
    concat_zeros = [np.zeros((n_cores * z.shape[0], *z.shape[1:]), z.dtype)
                    for z in zero_outs]
    sh = NamedSharding(mesh, PartitionSpec("core"))
    dev_in = [jax.device_put(a, sh) for a in concat_in]
    outs = sharded(*dev_in, *[jax.device_put(z, sh) for z in concat_zeros])
    jax.block_until_ready(outs)
    first = [np.asarray(o) for o in outs]

    times = []
    for _ in range(n_iters):
        t0 = time.perf_counter()
        outs = sharded(*dev_in, *outs)
        jax.block_until_ready(outs)
        times.append(time.perf_counter() - t0)

    results = [
        {name: first[i].reshape(n_cores, *out_avals[i].shape)[c]
         for i, name in enumerate(out_names)}
        for c in range(n_cores)
    ]
    return results, times


def _gather(results):
    out = np.zeros((B, T, D), dtype=np.float32)
    for c in range(N_CORES):
        out[c // 2] += np.asarray(results[c]["out"], dtype=np.float32)
    return out


def kernel(x, W_Q, W_K, W_V, W_O):
    import concourse.bass_utils as bass_utils

    x = np.asarray(x, dtype=np.float32)
    in_maps = make_in_maps(x, np.asarray(W_Q, np.float32),
                           np.asarray(W_K, np.float32),
                           np.asarray(W_V, np.float32),
                           np.asarray(W_O, np.float32))
    nc = _get_nc()
    trace = bool(int(os.environ.get("MHSA_TRACE", "0")))
    tmpdir = None
    if trace:
        trace = _ensure_ntff_hook()
    if trace:
        import tempfile
        tmpdir = tempfile.mkdtemp(prefix="mhsa_ntff_")
        _cache["trace_dir"] = tmpdir
        # no cloud creds in this container; keep artifacts local
        bass_utils.upload_artifacts = lambda d: f"local://{d}"
    res = bass_utils.run_bass_kernel_spmd(
        nc, in_maps, list(range(N_CORES)), trace=trace, tmpdir=tmpdir)
    _cache["last_results"] = res
    return _gather(res.results)


# revision 23
# speedup vs baseline: 1.6155x; 1.1926x over previous
"""Multi-head self-attention (B=4, T=2048, D=1024, H=16) on 8 TRN2 NeuronCores.

Sharding: batch x head-group. Core c owns batch b=c//2 and head group
g=c%2 (heads 8g..8g+7, i.e. model dims [512g, 512g+512)):
  - W_Q/W_K/W_V rows [512g, 512g+512) -> per-core q/k/v of shape [2048, 512]
  - causal attention for its 8 heads on its batch (block-skipped)
  - partial output projection through W_O columns [512g, 512g+512)
Host sums the 2 partial outputs per batch (row-parallel W_O reduction).

Layouts (on device, per core):
  x    [128, 8, 2048]   x^T for this batch, model dim on partitions (bf16)
  qT/kT [128, 4, 2048]  transposed q/k; head h lives at plane h//2,
                        rows 64*(h%2).. (bf16)
  vaug [128, 16, 520]   v token-major; per head [64 dims | ones col]
  scores^T tiles [128 k-tok, 2, 512 q] in PSUM; exp on ACT (bf16 out);
  causal masking is a post-exp 0/1 multiply on GpSimd (keeps DVE free);
  softmax denominator = ones-column row of the AV output; 1/denom via
  DVE reciprocal, broadcast across 64 partitions with a rank-1 matmul.

Emission interleaves projection chunk qc -> attention for q-chunk qc ->
output projection for those tokens, so ScalarE exp overlaps TensorE
projection matmuls and the PE stays HAM-warm.
"""

import os
import sys

import numpy as np

if "/opt/trn_rl_repo" not in sys.path:
    sys.path.insert(0, "/opt/trn_rl_repo")

import ml_dtypes

B, T, D, NH, DH = 4, 2048, 1024, 16, 64
MT = D // 128       # 8 model-dim tiles
N_CORES = 8
HPC = 8             # heads per core
DPC = 512           # model dims per core (head group)

_cache = {}


def _build_nc():
    from contextlib import ExitStack

    import concourse.mybir as mybir
    import concourse.tile as tile
    from concourse import bacc

    BF = mybir.dt.bfloat16
    F32 = mybir.dt.float32
    EXP = mybir.ActivationFunctionType.Exp
    LN = mybir.ActivationFunctionType.Ln

    nc = bacc.Bacc("TRN2", target_bir_lowering=False, debug=False)

    xT_d = nc.dram_tensor("xT", [MT, 128, T], BF, kind="ExternalInput")
    wq_d = nc.dram_tensor("wqT", [MT, 128, DPC], BF, kind="ExternalInput")
    wk_d = nc.dram_tensor("wkT", [MT, 128, DPC], BF, kind="ExternalInput")
    wv_d = nc.dram_tensor("wvT", [MT, 128, DPC], BF, kind="ExternalInput")
    wo_d = nc.dram_tensor("woT", [4, 128, D], BF, kind="ExternalInput")
    cm_d = nc.dram_tensor("cmask", [4, 128, 512], BF, kind="ExternalInput")
    out_d = nc.dram_tensor("out", [T, D], BF, kind="ExternalOutput")

    with tile.TileContext(nc) as tc, ExitStack() as ctx:
        pers = ctx.enter_context(tc.tile_pool(name="pers", bufs=1))
        xs = pers.tile([128, MT, T], BF)
        wq = pers.tile([128, MT, DPC], BF)
        wk = pers.tile([128, MT, DPC], BF)
        wv = pers.tile([128, MT, DPC], BF)
        wo = pers.tile([128, 4, D], BF)
        masks = pers.tile([128, 4, 512], BF)
        ones64 = pers.tile([1, 64], F32)
        qT = pers.tile([128, 4, T], BF)
        kT = pers.tile([128, 4, T], BF)
        vaug = pers.tile([128, 16, 520], BF)
        hoQ = [pers.tile([128, 4, 512], BF, tag=f"ho{qc}", name=f"ho{qc}")
               for qc in range(4)]

        nc.vector.memset(ones64, 1.0)
        nc.vector.memset(vaug, 1.0)
        for mt in range(MT):
            nc.sync.dma_start(out=wq[:, mt, :], in_=wq_d[mt])
            nc.sync.dma_start(out=wk[:, mt, :], in_=wk_d[mt])
            nc.sync.dma_start(out=wv[:, mt, :], in_=wv_d[mt])
        # x arrives chunk-by-chunk so chunk-0 projections start early
        for cc in range(4):
            for mt in range(MT):
                cs = slice(cc * 512, (cc + 1) * 512)
                nc.sync.dma_start(out=xs[:, mt, cs], in_=xT_d[mt, :, cs])
        for i in range(4):
            nc.sync.dma_start(out=masks[:, i, :], in_=cm_d[i])
            nc.sync.dma_start(out=wo[:, i, :], in_=wo_d[i])

        pp = ctx.enter_context(tc.tile_pool(name="pp", bufs=2, space="PSUM"))
        sp = ctx.enter_context(tc.tile_pool(name="ps_s", bufs=2, space="PSUM"))
        avp = ctx.enter_context(tc.tile_pool(name="ps_av", bufs=2, space="PSUM"))
        ep = ctx.enter_context(tc.tile_pool(name="esb", bufs=6))
        nrm = ctx.enter_context(tc.tile_pool(name="nrm", bufs=4))
        osb = ctx.enter_context(tc.tile_pool(name="osb", bufs=3))

        for qc in range(4):
            qs = slice(qc * 512, (qc + 1) * 512)
            # ---- projection for token chunk qc ----
            for dt in range(4):
                ds_ = slice(dt * 128, (dt + 1) * 128)
                pq = pp.tile([128, 512], F32, tag="pp")
                for mt in range(MT):
                    nc.tensor.matmul(pq, wq[:, mt, ds_], xs[:, mt, qs],
                                     start=(mt == 0), stop=(mt == MT - 1))
                nc.vector.tensor_copy(out=qT[:, dt, qs], in_=pq)
                pk = pp.tile([128, 512], F32, tag="pp")
                for mt in range(MT):
                    nc.tensor.matmul(pk, wk[:, mt, ds_], xs[:, mt, qs],
                                     start=(mt == 0), stop=(mt == MT - 1))
                nc.vector.tensor_copy(out=kT[:, dt, qs], in_=pk)
            for tt in range(4):
                ts_ = slice((qc * 4 + tt) * 128, (qc * 4 + tt + 1) * 128)
                pv = pp.tile([128, 512], F32, tag="pp")
                for mt in range(MT):
                    nc.tensor.matmul(pv, xs[:, mt, ts_], wv[:, mt, :],
                                     start=(mt == 0), stop=(mt == MT - 1))
                # scatter 8 heads' 64-col blocks into the 65-wide slots
                nc.vector.tensor_copy(
                    out=vaug[:, qc * 4 + tt, :].rearrange(
                        "p (h e) -> p h e", h=HPC)[:, :, 0:64],
                    in_=pv[:].rearrange("p (h e) -> p h e", h=HPC))

            # ---- causal attention for q-chunk qc, all 8 heads ----
            ns = 2 * (qc + 1)          # k-supertiles of 256 tokens
            for h in range(HPC):
                hp = 64 * (h % 2)
                dt = h // 2
                pavh = avp.tile([65, 512], F32, tag="pav")
                for s in range(ns):
                    pss = sp.tile([128, 2, 512], F32)
                    for j in range(2):
                        kt = 2 * s + j
                        ko = kt * 128
                        nc.tensor.matmul(
                            pss[:, j, :],
                            kT[hp:hp + 64, dt, ko:ko + 128],
                            qT[hp:hp + 64, dt, qs],
                            start=True, stop=True)
                    ex = ep.tile([128, 2, 512], BF)
                    nc.scalar.activation(out=ex[:], in_=pss[:], func=EXP,
                                         scale=0.125)
                    if s >= 2 * qc:    # diagonal band: zero future positions
                        j0 = 2 * (s - 2 * qc)
                        exm = ep.tile([128, 2, 512], BF, tag="exm")
                        nc.vector.tensor_mul(exm[:], ex[:],
                                             masks[:, j0:j0 + 2, :])
                        ex = exm
                    for j in range(2):
                        kt = 2 * s + j
                        nc.tensor.matmul(
                            pavh,
                            vaug[:, kt, 65 * h:65 * h + 65],
                            ex[:, j, :],
                            start=(s == 0 and j == 0),
                            stop=(s == ns - 1 and j == 1))
                # 1/denominator: single-pass DVE approx (18 bits, plenty
                # for the 2e-2 gate), broadcast across 64 partitions via a
                # rank-1 matmul, then scale the head output.
                dsb = nrm.tile([1, 512], F32)
                nc.vector.tensor_copy(out=dsb, in_=pavh[64:65, :])
                inv = nrm.tile([1, 512], F32)
                nc.vector.reciprocal_approx_fast(out=inv, in_=dsb)
                invb = nrm.tile([64, 512], F32)
                nc.gpsimd.partition_broadcast(invb, inv)
                nc.vector.tensor_mul(hoQ[qc][hp:hp + 64, dt, :],
                                     pavh[0:64, :], invb)

            # ---- partial output projection for tokens of chunk qc ----
            for tt in range(4):
                to = qc * 512 + tt * 128
                for oc in range(2):
                    po = pp.tile([128, 512], F32, tag="pp")
                    for dt in range(4):
                        nc.tensor.matmul(
                            po,
                            hoQ[qc][:, dt, tt * 128:(tt + 1) * 128],
                            wo[:, dt, oc * 512:(oc + 1) * 512],
                            start=(dt == 0), stop=(dt == 3))
                    ost = osb.tile([128, 512], BF)
                    nc.vector.tensor_copy(out=ost, in_=po)
                    nc.sync.dma_start(
                        out=out_d[to:to + 128, oc * 512:(oc + 1) * 512],
                        in_=ost)
    nc.compile()
    return nc


def _get_nc():
    if "nc" not in _cache:
        _cache["nc"] = _build_nc()
    return _cache["nc"]


def _bf(a):
    return np.ascontiguousarray(a, dtype=np.float32).astype(ml_dtypes.bfloat16)


def make_in_maps(x, W_Q, W_K, W_V, W_O):
    x = np.asarray(x, np.float32)
    cmask = np.zeros((4, 128, 512), dtype=np.float32)
    for t in range(4):
        for kp in range(128):
            cmask[t, kp, t * 128 + kp:] = 1.0
    cmask = cmask.astype(ml_dtypes.bfloat16)
    in_maps = []
    for c in range(N_CORES):
        b, g = c // 2, c % 2
        rs = slice(g * DPC, (g + 1) * DPC)
        in_maps.append({
            "xT": _bf(x[b].T).reshape(MT, 128, T),
            "wqT": _bf(W_Q[rs, :].T).reshape(MT, 128, DPC),
            "wkT": _bf(W_K[rs, :].T).reshape(MT, 128, DPC),
            "wvT": _bf(W_V[rs, :].T).reshape(MT, 128, DPC),
            "woT": _bf(W_O[:, rs].T).reshape(4, 128, D),
            "cmask": cmask,
        })
    return in_maps


def _ensure_ntff_hook():
    """Install antenv.axon_hooks shim (missing in this image) so
    run_bass_kernel_spmd(trace=True) can capture NTFF profiles."""
    try:
        from antenv import axon_hooks  # noqa: F401
        return True
    except ImportError:
        pass
    try:
        import contextlib
        import ctypes
        import types

        import antenv

        so_path = "/opt/axon/libaxon_pjrt.so"
        lib = ctypes.CDLL(so_path)
        if not hasattr(lib, "axon_start_nrt_profile"):
            return False
        lib.axon_start_nrt_profile.argtypes = [
            ctypes.POINTER(ctypes.c_int64), ctypes.c_size_t]
        lib.axon_start_nrt_profile.restype = ctypes.c_int64
        lib.axon_stop_nrt_profile.argtypes = [ctypes.c_char_p]
        lib.axon_stop_nrt_profile.restype = ctypes.c_int64

        @contextlib.contextmanager
        def _hook(output_dir, device_ids):
            import jax
            jax.devices()
            if device_ids:
                ids = (ctypes.c_int64 * len(device_ids))(*device_ids)
                rc = lib.axon_start_nrt_profile(ids, len(device_ids))
            else:
                rc = lib.axon_start_nrt_profile(None, 0)
            if rc != 0:
                raise RuntimeError(f"axon_start_nrt_profile rc={rc}")
            try:
                yield
            finally:
                n = lib.axon_stop_nrt_profile(str(output_dir).encode())
                print(f"ntff profile: {n} file(s) -> {output_dir}",
                      file=sys.stderr)

        mod = types.ModuleType("antenv.axon_hooks")
        mod.get_axon_ntff_profile_hook = lambda: _hook
        mod.set_axon_ntff_profile_hook = lambda h: None
        sys.modules["antenv.axon_hooks"] = mod
        antenv.axon_hooks = mod
        return True
    except Exception as e:  # pragma: no cover
        print(f"ntff hook install failed: {e}", file=sys.stderr)
        return False


def bench_pjrt(in_maps, n_iters=8):
    """Run the SPMD program with device-resident inputs; return (results,
    per-iter wall times)."""
    import time

    import jax
    import concourse.mybir as mybir
    from jax.sharding import Mesh, NamedSharding, PartitionSpec
    from jax.experimental.shard_map import shard_map
    from concourse import bass2jax

    nc = _get_nc()
    bass2jax.install_neuronx_cc_hook()

    part_name = nc.partition_id_tensor.name if nc.partition_id_tensor else None
    in_names, out_names, out_avals, zero_outs = [], [], [], []
    for alloc in nc.m.functions[0].allocations:
        if not isinstance(alloc, mybir.MemoryLocationSet):
            continue
        name = alloc.memorylocations[0].name
        if alloc.kind == "ExternalInput":
            if name != part_name:
                in_names.append(name)
        elif alloc.kind == "ExternalOutput":
            shape = tuple(alloc.tensor_shape)
            dtype = mybir.dt.np(alloc.dtype)
            out_names.append(name)
            out_avals.append(jax.core.ShapedArray(shape, dtype))
            zero_outs.append(np.zeros(shape, dtype))
    n_params = len(in_names)
    all_names = in_names + out_names
    if part_name is not None:
        all_names = all_names + [part_name]

    def _body(*args):
        operands = list(args)
        if part_name is not None:
            operands.append(bass2jax.partition_id_tensor())
        outs = bass2jax._bass_exec_p.bind(
            *operands,
            out_avals=tuple(out_avals),
            in_names=tuple(all_names),
            out_names=tuple(out_names),
            lowering_input_output_aliases=(),
            sim_require_finite=True,
            sim_require_nnan=True,
            nc=nc,
        )
        return tuple(outs)

    n_cores = len(in_maps)
    devices = jax.devices()[:n_cores]
    mesh = Mesh(np.asarray(devices), ("core",))
    donate = tuple(range(n_params, n_params + len(out_names)))
    sharded = jax.jit(
        shard_map(_body, mesh=mesh,
                  in_specs=(PartitionSpec("core"),) * (n_params + len(out_names)),
                  out_specs=(PartitionSpec("core"),) * len(out_names),
                  check_rep=False),
        donate_argnums=donate, keep_unused=True)

    concat_in = [
        np.concatenate([np.asarray(in_maps[c][k]) for c in range(n_cores)],
                       axis=0) for k in in_names]
    concat_zeros = [np.zeros((n_cores * z.shape[0], *z.shape[1:]), z.dtype)
                    for z in zero_outs]
    sh = NamedSharding(mesh, PartitionSpec("core"))
    dev_in = [jax.device_put(a, sh) for a in concat_in]
    outs = sharded(*dev_in, *[jax.device_put(z, sh) for z in concat_zeros])
    jax.block_until_ready(outs)
    first = [np.asarray(o) for o in outs]

    times = []
    for _ in range(n_iters):
        t0 = time.perf_counter()
        outs = sharded(*dev_in, *outs)
        jax.block_until_ready(outs)
        times.append(time.perf_counter() - t0)

    results = [
        {name: first[i].reshape(n_cores, *out_avals[i].shape)[c]
         for i, name in enumerate(out_names)}
        for c in range(n_cores)
    ]
    return results, times


def _gather(results):
    out = np.zeros((B, T, D), dtype=np.float32)
    for c in range(N_CORES):
        out[c // 2] += np.asarray(results[c]["out"], dtype=np.float32)
    return out


def kernel(x, W_Q, W_K, W_V, W_O):
    import concourse.bass_utils as bass_utils

    x = np.asarray(x, dtype=np.float32)
    in_maps = make_in_maps(x, np.asarray(W_Q, np.float32),
                           np.asarray(W_K, np.float32),
                           np.asarray(W_V, np.float32),
                           np.asarray(W_O, np.float32))
    nc = _get_nc()
    trace = bool(int(os.environ.get("MHSA_TRACE", "0")))
    tmpdir = None
    if trace:
        trace = _ensure_ntff_hook()
    if trace:
        import tempfile
        tmpdir = tempfile.mkdtemp(prefix="mhsa_ntff_")
        _cache["trace_dir"] = tmpdir
        # no cloud creds in this container; keep artifacts local
        bass_utils.upload_artifacts = lambda d: f"local://{d}"
    res = bass_utils.run_bass_kernel_spmd(
        nc, in_maps, list(range(N_CORES)), trace=trace, tmpdir=tmpdir)
    _cache["last_results"] = res
    return _gather(res.results)


# revision 27
# speedup vs baseline: 1.8733x; 1.1595x over previous
"""Multi-head self-attention (B=4, T=2048, D=1024, H=16) on 8 TRN2 NeuronCores.

Sharding: batch x head-group. Core c owns batch b=c//2 and head group
g=c%2 (heads 8g..8g+7, i.e. model dims [512g, 512g+512)):
  - W_Q/W_K/W_V rows [512g, 512g+512) -> per-core q/k/v of shape [2048, 512]
  - causal attention for its 8 heads on its batch (block-skipped)
  - partial output projection through W_O columns [512g, 512g+512)
Host sums the 2 partial outputs per batch (row-parallel W_O reduction).

Layouts (on device, per core):
  x    [128, 8, 2048]   x^T for this batch, model dim on partitions (bf16)
  qT/kT [128, 4, 2048]  transposed q/k; head h lives at plane h//2,
                        rows 64*(h%2).. (bf16)
  vaug [128, 16, 520]   v token-major; per head [64 dims | ones col]
  scores^T tiles [128 k-tok, 2, 512 q] in PSUM; exp on ACT (bf16 out);
  causal masking is a post-exp 0/1 multiply on GpSimd (keeps DVE free);
  softmax denominator = ones-column row of the AV output; 1/denom via
  DVE reciprocal, broadcast across 64 partitions with a rank-1 matmul.

Emission interleaves projection chunk qc -> attention for q-chunk qc ->
output projection for those tokens, so ScalarE exp overlaps TensorE
projection matmuls and the PE stays HAM-warm.
"""

import os
import sys

import numpy as np

if "/opt/trn_rl_repo" not in sys.path:
    sys.path.insert(0, "/opt/trn_rl_repo")

import ml_dtypes

B, T, D, NH, DH = 4, 2048, 1024, 16, 64
MT = D // 128       # 8 model-dim tiles
N_CORES = 8
HPC = 8             # heads per core
DPC = 512           # model dims per core (head group)

_cache = {}


def _build_nc():
    from contextlib import ExitStack

    import concourse.mybir as mybir
    import concourse.tile as tile
    from concourse import bacc

    BF = mybir.dt.bfloat16
    F32 = mybir.dt.float32
    EXP = mybir.ActivationFunctionType.Exp
    LN = mybir.ActivationFunctionType.Ln

    nc = bacc.Bacc("TRN2", target_bir_lowering=False, debug=False)

    xT_d = nc.dram_tensor("xT", [MT, 128, T], BF, kind="ExternalInput")
    wq_d = nc.dram_tensor("wqT", [MT, 128, DPC], BF, kind="ExternalInput")
    wk_d = nc.dram_tensor("wkT", [MT, 128, DPC], BF, kind="ExternalInput")
    wv_d = nc.dram_tensor("wvT", [MT, 128, DPC], BF, kind="ExternalInput")
    wo_d = nc.dram_tensor("woT", [4, 128, D], BF, kind="ExternalInput")
    cm_d = nc.dram_tensor("cmask", [4, 128, 512], BF, kind="ExternalInput")
    out_d = nc.dram_tensor("out", [T, D], BF, kind="ExternalOutput")

    with tile.TileContext(nc) as tc, ExitStack() as ctx:
        pers = ctx.enter_context(tc.tile_pool(name="pers", bufs=1))
        xs = pers.tile([128, MT, T], BF)
        wq = pers.tile([128, MT, DPC], BF)
        wk = pers.tile([128, MT, DPC], BF)
        wv = pers.tile([128, MT, DPC], BF)
        wo = pers.tile([128, 4, D], BF)
        masks = pers.tile([128, 4, 512], BF)
        ones64 = pers.tile([1, 64], F32)
        qT = pers.tile([128, 4, T], BF)
        kT = pers.tile([128, 4, T], BF)
        vaug = pers.tile([128, 16, 520], BF)
        hoQ = [pers.tile([128, 4, 512], BF, tag=f"ho{qc}", name=f"ho{qc}")
               for qc in range(4)]

        nc.vector.memset(ones64, 1.0)
        nc.vector.memset(vaug, 1.0)
        # DMA order tracks first use: wq + x chunk 0 unblock the q
        # projection, then wk, wv, the rest of x, and lastly wo/masks.
        for mt in range(MT):
            nc.sync.dma_start(out=wq[:, mt, :], in_=wq_d[mt])
        for mt in range(MT):
            nc.sync.dma_start(out=xs[:, mt, 0:512], in_=xT_d[mt, :, 0:512])
        for mt in range(MT):
            nc.sync.dma_start(out=wk[:, mt, :], in_=wk_d[mt])
        for mt in range(MT):
            nc.sync.dma_start(out=wv[:, mt, :], in_=wv_d[mt])
        for i in range(4):
            nc.sync.dma_start(out=masks[:, i, :], in_=cm_d[i])
        for cc in range(1, 4):
            for mt in range(MT):
                cs = slice(cc * 512, (cc + 1) * 512)
                nc.sync.dma_start(out=xs[:, mt, cs], in_=xT_d[mt, :, cs])
        for i in range(4):
            nc.sync.dma_start(out=wo[:, i, :], in_=wo_d[i])

        pp = ctx.enter_context(tc.tile_pool(name="pp", bufs=2, space="PSUM"))
        sp = ctx.enter_context(tc.tile_pool(name="ps_s", bufs=2, space="PSUM"))
        avp = ctx.enter_context(tc.tile_pool(name="ps_av", bufs=2, space="PSUM"))
        ep = ctx.enter_context(tc.tile_pool(name="esb", bufs=6))
        nrm = ctx.enter_context(tc.tile_pool(name="nrm", bufs=4))
        osb = ctx.enter_context(tc.tile_pool(name="osb", bufs=3))

        def emit_qk(cc, dt):
            cs = slice(cc * 512, (cc + 1) * 512)
            ds_ = slice(dt * 128, (dt + 1) * 128)
            pq = pp.tile([128, 512], F32, tag="pp")
            for mt in range(MT):
                nc.tensor.matmul(pq, wq[:, mt, ds_], xs[:, mt, cs],
                                 start=(mt == 0), stop=(mt == MT - 1))
            nc.vector.tensor_copy(out=qT[:, dt, cs], in_=pq)
            pk = pp.tile([128, 512], F32, tag="pp")
            for mt in range(MT):
                nc.tensor.matmul(pk, wk[:, mt, ds_], xs[:, mt, cs],
                                 start=(mt == 0), stop=(mt == MT - 1))
            nc.vector.tensor_copy(out=kT[:, dt, cs], in_=pk)

        def emit_v(cc, tt):
            ts_ = slice((cc * 4 + tt) * 128, (cc * 4 + tt + 1) * 128)
            pv = pp.tile([128, 512], F32, tag="pp")
            for mt in range(MT):
                nc.tensor.matmul(pv, xs[:, mt, ts_], wv[:, mt, :],
                                 start=(mt == 0), stop=(mt == MT - 1))
            # scatter 8 heads' 64-col blocks into the 65-wide slots
            nc.vector.tensor_copy(
                out=vaug[:, cc * 4 + tt, :].rearrange(
                    "p (h e) -> p h e", h=HPC)[:, :, 0:64],
                in_=pv[:].rearrange("p (h e) -> p h e", h=HPC))

        def emit_proj_blocks(cc):
            return ([lambda dt=dt: emit_qk(cc, dt) for dt in range(4)] +
                    [lambda tt=tt: emit_v(cc, tt) for tt in range(4)])

        # chunk-0 projections up front; later chunks interleave with the
        # previous chunk's attention so TensorE never starves
        for blk in emit_proj_blocks(0):
            blk()

        for qc in range(4):
            qs = slice(qc * 512, (qc + 1) * 512)
            nxt = emit_proj_blocks(qc + 1) if qc < 3 else []

            # ---- causal attention for q-chunk qc, all 8 heads ----
            ns = 2 * (qc + 1)          # k-supertiles of 256 tokens
            for h in range(HPC):
                for bi in range(len(nxt) * h // HPC,
                                len(nxt) * (h + 1) // HPC):
                    nxt[bi]()
                hp = 64 * (h % 2)
                dt = h // 2
                pavh = avp.tile([65, 512], F32, tag="pav")
                for s in range(ns):
                    pss = sp.tile([128, 2, 512], F32)
                    for j in range(2):
                        kt = 2 * s + j
                        ko = kt * 128
                        nc.tensor.matmul(
                            pss[:, j, :],
                            kT[hp:hp + 64, dt, ko:ko + 128],
                            qT[hp:hp + 64, dt, qs],
                            start=True, stop=True)
                    ex = ep.tile([128, 2, 512], BF)
                    nc.scalar.activation(out=ex[:], in_=pss[:], func=EXP,
                                         scale=0.125)
                    if s >= 2 * qc:    # diagonal band: zero future positions
                        j0 = 2 * (s - 2 * qc)
                        exm = ep.tile([128, 2, 512], BF, tag="exm")
                        nc.vector.tensor_mul(exm[:], ex[:],
                                             masks[:, j0:j0 + 2, :])
                        ex = exm
                    for j in range(2):
                        kt = 2 * s + j
                        nc.tensor.matmul(
                            pavh,
                            vaug[:, kt, 65 * h:65 * h + 65],
                            ex[:, j, :],
                            start=(s == 0 and j == 0),
                            stop=(s == ns - 1 and j == 1))
                # 1/denominator: single-pass DVE approx (18 bits, plenty
                # for the 2e-2 gate), broadcast across 64 partitions via a
                # rank-1 matmul, then scale the head output.
                dsb = nrm.tile([1, 512], F32)
                nc.vector.tensor_copy(out=dsb, in_=pavh[64:65, :])
                inv = nrm.tile([1, 512], F32)
                nc.vector.reciprocal_approx_fast(out=inv, in_=dsb)
                invb = nrm.tile([64, 512], F32)
                nc.gpsimd.partition_broadcast(invb, inv)
                nc.vector.tensor_mul(hoQ[qc][hp:hp + 64, dt, :],
                                     pavh[0:64, :], invb)

            # ---- partial output projection for tokens of chunk qc ----
            for tt in range(4):
                to = qc * 512 + tt * 128
                for oc in range(2):
                    po = pp.tile([128, 512], F32, tag="pp")
                    for dt in range(4):
                        nc.tensor.matmul(
                            po,
                            hoQ[qc][:, dt, tt * 128:(tt + 1) * 128],
                            wo[:, dt, oc * 512:(oc + 1) * 512],
                            start=(dt == 0), stop=(dt == 3))
                    ost = osb.tile([128, 512], BF)
                    nc.vector.tensor_copy(out=ost, in_=po)
                    nc.sync.dma_start(
                        out=out_d[to:to + 128, oc * 512:(oc + 1) * 512],
                        in_=ost)
    nc.compile()
    return nc


def _get_nc():
    if "nc" not in _cache:
        _cache["nc"] = _build_nc()
    return _cache["nc"]


def _bf(a):
    return np.ascontiguousarray(a, dtype=np.float32).astype(ml_dtypes.bfloat16)


def make_in_maps(x, W_Q, W_K, W_V, W_O):
    x = np.asarray(x, np.float32)
    cmask = np.zeros((4, 128, 512), dtype=np.float32)
    for t in range(4):
        for kp in range(128):
            cmask[t, kp, t * 128 + kp:] = 1.0
    cmask = cmask.astype(ml_dtypes.bfloat16)
    in_maps = []
    for c in range(N_CORES):
        b, g = c // 2, c % 2
        rs = slice(g * DPC, (g + 1) * DPC)
        in_maps.append({
            "xT": _bf(x[b].T).reshape(MT, 128, T),
            "wqT": _bf(W_Q[rs, :].T).reshape(MT, 128, DPC),
            "wkT": _bf(W_K[rs, :].T).reshape(MT, 128, DPC),
            "wvT": _bf(W_V[rs, :].T).reshape(MT, 128, DPC),
            "woT": _bf(W_O[:, rs].T).reshape(4, 128, D),
            "cmask": cmask,
        })
    return in_maps


def _ensure_ntff_hook():
    """Install antenv.axon_hooks shim (missing in this image) so
    run_bass_kernel_spmd(trace=True) can capture NTFF profiles."""
    try:
        from antenv import axon_hooks  # noqa: F401
        return True
    except ImportError:
        pass
    try:
        import contextlib
        import ctypes
        import types

        import antenv

        so_path = "/opt/axon/libaxon_pjrt.so"
        lib = ctypes.CDLL(so_path)
        if not hasattr(lib, "axon_start_nrt_profile"):
            return False
        lib.axon_start_nrt_profile.argtypes = [
            ctypes.POINTER(ctypes.c_int64), ctypes.c_size_t]
        lib.axon_start_nrt_profile.restype = ctypes.c_int64
        lib.axon_stop_nrt_profile.argtypes = [ctypes.c_char_p]
        lib.axon_stop_nrt_profile.restype = ctypes.c_int64

        @contextlib.contextmanager
        def _hook(output_dir, device_ids):
            import jax
            jax.devices()
            if device_ids:
                ids = (ctypes.c_int64 * len(device_ids))(*device_ids)
                rc = lib.axon_start_nrt_profile(ids, len(device_ids))
            else:
                rc = lib.axon_start_nrt_profile(None, 0)
            if rc != 0:
                raise RuntimeError(f"axon_start_nrt_profile rc={rc}")
            try:
                yield
            finally:
                n = lib.axon_stop_nrt_profile(str(output_dir).encode())
                print(f"ntff profile: {n} file(s) -> {output_dir}",
                      file=sys.stderr)

        mod = types.ModuleType("antenv.axon_hooks")
        mod.get_axon_ntff_profile_hook = lambda: _hook
        mod.set_axon_ntff_profile_hook = lambda h: None
        sys.modules["antenv.axon_hooks"] = mod
        antenv.axon_hooks = mod
        return True
    except Exception as e:  # pragma: no cover
        print(f"ntff hook install failed: {e}", file=sys.stderr)
        return False


def bench_pjrt(in_maps, n_iters=8, return_last=False):
    """Run the SPMD program with device-resident inputs; return (results,
    per-iter wall times)."""
    import time

    import jax
    import concourse.mybir as mybir
    from jax.sharding import Mesh, NamedSharding, PartitionSpec
    from jax.experimental.shard_map import shard_map
    from concourse import bass2jax

    nc = _get_nc()
    bass2jax.install_neuronx_cc_hook()

    part_name = nc.partition_id_tensor.name if nc.partition_id_tensor else None
    in_names, out_names, out_avals, zero_outs = [], [], [], []
    for alloc in nc.m.functions[0].allocations:
        if not isinstance(alloc, mybir.MemoryLocationSet):
            continue
        name = alloc.memorylocations[0].name
        if alloc.kind == "ExternalInput":
            if name != part_name:
                in_names.append(name)
        elif alloc.kind == "ExternalOutput":
            shape = tuple(alloc.tensor_shape)
            dtype = mybir.dt.np(alloc.dtype)
            out_names.append(name)
            out_avals.append(jax.core.ShapedArray(shape, dtype))
            zero_outs.append(np.zeros(shape, dtype))
    n_params = len(in_names)
    all_names = in_names + out_names
    if part_name is not None:
        all_names = all_names + [part_name]

    def _body(*args):
        operands = list(args)
        if part_name is not None:
            operands.append(bass2jax.partition_id_tensor())
        outs = bass2jax._bass_exec_p.bind(
            *operands,
            out_avals=tuple(out_avals),
            in_names=tuple(all_names),
            out_names=tuple(out_names),
            lowering_input_output_aliases=(),
            sim_require_finite=True,
            sim_require_nnan=True,
            nc=nc,
        )
        return tuple(outs)

    n_cores = len(in_maps)
    devices = jax.devices()[:n_cores]
    mesh = Mesh(np.asarray(devices), ("core",))
    donate = tuple(range(n_params, n_params + len(out_names)))
    sharded = jax.jit(
        shard_map(_body, mesh=mesh,
                  in_specs=(PartitionSpec("core"),) * (n_params + len(out_names)),
                  out_specs=(PartitionSpec("core"),) * len(out_names),
                  check_rep=False),
        donate_argnums=donate, keep_unused=True)

    concat_in = [
        np.concatenate([np.asarray(in_maps[c][k]) for c in range(n_cores)],
                       axis=0) for k in in_names]
    concat_zeros = [np.zeros((n_cores * z.shape[0], *z.shape[1:]), z.dtype)
                    for z in zero_outs]
    sh = NamedSharding(mesh, PartitionSpec("core"))
    dev_in = [jax.device_put(a, sh) for a in concat_in]
    outs = sharded(*dev_in, *[jax.device_put(z, sh) for z in concat_zeros])
    jax.block_until_ready(outs)
    first = [np.asarray(o) for o in outs]

    times = []
    for _ in range(n_iters):
        t0 = time.perf_counter()
        outs = sharded(*dev_in, *outs)
        jax.block_until_ready(outs)
        times.append(time.perf_counter() - t0)

    results = [
        {name: first[i].reshape(n_cores, *out_avals[i].shape)[c]
         for i, name in enumerate(out_names)}
        for c in range(n_cores)
    ]
    if return_last:
        fin = [np.asarray(o) for o in outs]
        last = [
            {name: fin[i].reshape(n_cores, *out_avals[i].shape)[c]
             for i, name in enumerate(out_names)}
            for c in range(n_cores)
        ]
        return results, times, last
    return results, times


def _gather(results):
    out = np.zeros((B, T, D), dtype=np.float32)
    for c in range(N_CORES):
        out[c // 2] += np.asarray(results[c]["out"], dtype=np.float32)
    return out


def kernel(x, W_Q, W_K, W_V, W_O):
    import concourse.bass_utils as bass_utils

    x = np.asarray(x, dtype=np.float32)
    in_maps = make_in_maps(x, np.asarray(W_Q, np.float32),
                           np.asarray(W_K, np.float32),
                           np.asarray(W_V, np.float32),
                           np.asarray(W_O, np.float32))
    nc = _get_nc()
    trace = bool(int(os.environ.get("MHSA_TRACE", "0")))
    tmpdir = None
    if trace:
        trace = _ensure_ntff_hook()
    if trace:
        import tempfile
        tmpdir = tempfile.mkdtemp(prefix="mhsa_ntff_")
        _cache["trace_dir"] = tmpdir
        # no cloud creds in this container; keep artifacts local
        bass_utils.upload_artifacts = lambda d: f"local://{d}"
    res = bass_utils.run_bass_kernel_spmd(
        nc, in_maps, list(range(N_CORES)), trace=trace, tmpdir=tmpdir)
    _cache["last_results"] = res
    return _gather(res.results)
